# revision 1
# baseline (speedup 1.0000x reference)
"""MoE layer (B=4,S=2048,H=1024,F=4096,E=8,K=2) on 8 Trainium2 NeuronCores.

Strategy: expert-parallel. The gate (0.1% of FLOPs) + top-2 routing run on
host; tokens are gathered per expert and each of the 8 cores runs one
expert's dense FFN  y = relu(x@w1+b1)@w2+b2  over its routed tokens in
float32r (full-rate fp32 matmul mode on TRN2). The host applies the combine
weights and scatter-adds the two expert contributions per token.

Per token block, the two matmuls are interleaved at f-chunk granularity:
  A(f): hT[f] = relu(w1[:,f]^T @ xgt + b1[f])   (x chunk moving, 384 tokens)
  B(f): y[tt,hb] += hT[f,tt]^T @ w2[f, hb]      (w2 moving, 512 wide)
so the w1 stream (the dominant DMA traffic, re-fetched per block) is spread
evenly over the whole timeline instead of saturating HBM during a separate
stage-A phase. w2 stays resident in SBUF (16.8MB, loaded during block 0's
f-loop). y accumulates token-major in 6 PSUM banks per block and is written
out token-major.
"""

import numpy as np

B, S, H, F, E, TOPK = 4, 2048, 1024, 4096, 8, 2
T = B * S
C = 2048          # per-expert device capacity: exactly 16 128-token tiles, so
#                   matmul2 pays zero tile padding. Seed-0 expert loads are
#                   1932..2182; the ~291 overflow tokens (1.8% of routed
#                   pairs) run through the exact host-side fp32 fallback
#                   below, as in standard MoE capacity-factor designs (but
#                   computed exactly instead of dropped).
TB = 384          # token block (moving dim of matmul1; >=256 keeps fp32r full rate)
BLOCKS = [(i * TB, TB) for i in range(4)] + [(4 * TB, 256), (4 * TB + 256, 256)]
NF = F // 128     # 32 F-chunks
KH = H // 128     # 8 H-chunks (contraction for matmul1)
NH = H // 128     # 8 H-chunks
HB = H // 512     # 2 output column halves of matmul2 (512 = fp32 moving max)

_NC_CACHE = {}


def _build_nc():
    import concourse.bacc as bacc
    import concourse.mybir as mybir
    from concourse.tile import TileContext

    f32 = mybir.dt.float32
    f32r = mybir.dt.float32r
    Relu = mybir.ActivationFunctionType.Relu

    nc = bacc.Bacc("TRN2", target_bir_lowering=False, debug=False, num_devices=E,
                   dynamic_dma_scratch_size=4096)
    xgt = nc.declare_dram_parameter("xgt", [H, C], f32r, isOutput=False)
    w1t = nc.declare_dram_parameter("w1t", [128, NF, KH, 128], f32r, isOutput=False)
    w2t = nc.declare_dram_parameter("w2t", [128, NF, H], f32r, isOutput=False)
    b1t = nc.declare_dram_parameter("b1t", [128, NF], f32, isOutput=False)
    out = nc.declare_dram_parameter("out", [C, H], f32, isOutput=True)     # token-major

    xgt_r = xgt.rearrange("(k p) c -> k p c", p=128)

    with TileContext(nc) as tc:
        with tc.tile_pool(name="res", bufs=1) as res_pool, \
             tc.tile_pool(name="xp", bufs=16) as x_pool, \
             tc.tile_pool(name="w1p", bufs=3) as w1_pool, \
             tc.tile_pool(name="hp", bufs=1) as h_pool, \
             tc.tile_pool(name="yp", bufs=2) as y_pool, \
             tc.tile_pool(name="p1", bufs=2, space="PSUM") as p1_pool, \
             tc.tile_pool(name="py", bufs=1, space="PSUM") as py_pool:
            # Resident across the whole kernel: full w2 (16.8MB) + biases.
            # w2 chunk f's DMA is issued inside block 0's f-loop right before
            # its first use, overlapping the load with block-0 compute.
            w2s = res_pool.tile([128, NF, H], f32r)
            b1s = res_pool.tile([128, NF], f32)

            w1s0 = w1_pool.tile([128, KH, 128], f32r, tag="w1s", name="w1s0")
            for k0 in range(KH):
                nc.sync.dma_start(out=w1s0[:, k0, :], in_=w1t[:, 0, k0, :])
            for b, (t0, tb) in enumerate(BLOCKS):
                blk = slice(t0, t0 + tb)
                ntt = (tb + 127) // 128
                xs = []
                for k in range(KH):
                    xk = x_pool.tile([128, TB], f32r, tag="xs")
                    nc.sync.dma_start(out=xk[:, :tb], in_=xgt_r[k][:, blk])
                    xs.append(xk)
                if b == 0:
                    nc.sync.dma_start(out=b1s[:], in_=b1t[:])
                    nc.sync.dma_start(out=w2s[:, 0, :], in_=w2t[:, 0, :])
                hs = h_pool.tile([128, NF, TB], f32r, tag="hs")
                pys = [[py_pool.tile([128, 512], f32, tag=f"py{tt}_{hb}",
                                     name=f"py_{b}_{tt}_{hb}")
                        for hb in range(HB)] for tt in range(ntt)]
                def emit_b(f):
                    # matmul2 partial for chunk f: y[tt,hb] += hs[f,tt]^T @ w2[f,hb]
                    for tt in range(ntt):
                        m = min(128, tb - tt * 128)
                        hsf = hs[:, f, tt * 128:tt * 128 + m]
                        for hb in range(HB):
                            nc.tensor.matmul(
                                pys[tt][hb][:m, :], hsf,
                                w2s[:, f, hb * 512:(hb + 1) * 512],
                                start=(f == 0), stop=(f == NF - 1),
                            )

                # Software-pipelined by one f: emit B(f-1) after A(f)'s
                # matmuls, so B never waits on the ACT that produces hs[f]
                # (it finished during A(f)'s eight matmuls).
                for f in range(NF):
                    if b == 0 and f == 0:
                        w1s = w1s0
                    else:
                        w1s = w1_pool.tile([128, KH, 128], f32r, tag="w1s")
                        nc.sync.dma_start(out=w1s[:], in_=w1t[:, f])
                    p1 = p1_pool.tile([128, TB], f32, tag="p1")
                    for k in range(KH):
                        nc.tensor.matmul(
                            p1[:, :tb], w1s[:, k, :], xs[k][:, :tb],
                            start=(k == 0), stop=(k == KH - 1),
                        )
                    nc.scalar.activation(hs[:, f, :tb], p1[:, :tb], Relu,
                                         bias=b1s[:, f:f + 1])
                    if b == 0 and f + 1 < NF:
                        # prefetch the NEXT w2 chunk: consumed by B(f+1) two
                        # f-iterations from now, doubling its arrival slack
                        nc.sync.dma_start(out=w2s[:, f + 1, :], in_=w2t[:, f + 1, :])
                    if f > 0:
                        emit_b(f - 1)
                emit_b(NF - 1)
                for tt in range(ntt):
                    m = min(128, tb - tt * 128)
                    for hb in range(HB):
                        ys = y_pool.tile([128, 512], f32, tag="ys")
                        nc.vector.tensor_copy(ys[:m, :], pys[tt][hb][:m, :])
                        nc.sync.dma_start(
                            out=out[t0 + tt * 128:t0 + tt * 128 + m,
                                    hb * 512:(hb + 1) * 512],
                            in_=ys[:m, :])
    nc.compile()
    return nc


def _get_nc():
    if "nc" not in _NC_CACHE:
        _NC_CACHE["nc"] = _build_nc()
    return _NC_CACHE["nc"]


def _route(xf, gate_w, gate_b):
    """Top-2 gating identical to softmax+top_k+renorm (softmax is monotonic,
    and the softmax denominator cancels in the renormalization)."""
    z = xf @ gate_w + gate_b                      # [T, E] f32
    rows = np.arange(T)
    i1 = z.argmax(1)
    z2 = z.copy()
    z2[rows, i1] = -np.inf
    i2 = z2.argmax(1)
    d = np.exp((z[rows, i2] - z[rows, i1]).astype(np.float32))
    c1 = (1.0 / (1.0 + d)).astype(np.float32)
    c2 = (1.0 - c1).astype(np.float32)
    return i1, i2, c1, c2


def kernel(x, gate_w, gate_b, w1, b1, w2, b2):
    import os
    try:  # pragma: no cover - env probe
        from antenv.axon_hooks import get_axon_ntff_profile_hook  # noqa: F401
    except ImportError:
        # BASS_TRACE=1 in the environment would send run_bass_kernel_spmd
        # down the NTFF-profiling path, which hard-imports antenv.axon_hooks.
        # If that module is absent, disable tracing rather than crash.
        os.environ.setdefault("BASS_NEVER_TRACE", "1")
    from concourse.bass_utils import run_bass_kernel_spmd

    xf = np.ascontiguousarray(np.asarray(x, dtype=np.float32).reshape(T, H))
    gate_w = np.asarray(gate_w, dtype=np.float32)
    gate_b = np.asarray(gate_b, dtype=np.float32)
    w1 = np.asarray(w1, dtype=np.float32)
    b1 = np.asarray(b1, dtype=np.float32)
    w2 = np.asarray(w2, dtype=np.float32)
    b2 = np.asarray(b2, dtype=np.float32)

    i1, i2, c1, c2 = _route(xf, gate_w, gate_b)

    in_maps = []
    scatter = []
    overflow = []
    for e in range(E):
        m1 = i1 == e
        m2 = i2 == e
        idx = np.concatenate([np.nonzero(m1)[0], np.nonzero(m2)[0]])
        wgt = np.concatenate([c1[m1], c2[m2]]).astype(np.float32)
        cnt = idx.size
        if cnt > C:
            # Capacity overflow (cannot happen for the fixed seed-0 inputs,
            # where the max expert load is 2182): compute the overflow rows
            # exactly on host so the result stays correct for any input.
            oidx, owgt = idx[C:], wgt[C:]
            h = np.maximum(xf[oidx] @ w1[e] + b1[e], 0.0)
            overflow.append((oidx, owgt, h @ w2[e] + b2[e]))
            idx, wgt, cnt = idx[:C], wgt[:C], C
        xg = np.zeros((C, H), np.float32)
        xg[:cnt] = xf[idx]
        xgt = np.ascontiguousarray(xg.T)                                    # [H, C]
        w1e = np.ascontiguousarray(
            w1[e].reshape(KH, 128, NF, 128).transpose(1, 2, 0, 3))          # [128,NF,KH,128]
        w2e = np.ascontiguousarray(w2[e].reshape(NF, 128, H).transpose(1, 0, 2))  # [128,NF,H]
        b1e = np.ascontiguousarray(b1[e].reshape(NF, 128).T)                # [128,NF]
        in_maps.append({"xgt": xgt, "w1t": w1e, "w2t": w2e, "b1t": b1e})
        scatter.append((idx, wgt, cnt))

    nc = _get_nc()
    res = run_bass_kernel_spmd(nc, in_maps, core_ids=list(range(E)))

    outf = np.zeros((T, H), np.float32)
    for e in range(E):
        idx, wgt, cnt = scatter[e]
        ye = res.results[e]["out"]                                          # [C, H]
        outf[idx] += (ye[:cnt] + b2[e]) * wgt[:, None]
    for oidx, owgt, oy in overflow:
        outf[oidx] += oy * owgt[:, None]
    return outf.reshape(B, S, H)



# revision 10
# speedup vs baseline: 1.0197x; 1.0197x over previous
"""MoE layer (B=4,S=2048,H=1024,F=4096,E=8,K=2) on 8 Trainium2 NeuronCores.

Expert-parallel: the gate (0.1% of FLOPs) + top-2 routing run on host;
tokens are gathered per expert and each of the 8 cores runs one expert's
dense FFN  y = relu(x@w1+b1)@w2+b2  over its C=2048 routed tokens, in fp16
(~1e-3 rel err, well inside the 2e-2 gate; full PE rate, same 1 cycle/row
as bf16/fp32r). The host applies combine weights and scatter-adds the two
expert contributions per token.

v2 structure (baseline fp32r kernel measured 567us on HW, PE busy 89.5%
but matmul spacing throttled by per-matmul 187ns LDWEIGHTS > 160ns row
time of its 384-row m1 matmuls, plus ~100MB/core of re-streamed w1):
- tokens run in 2 chunks of 1024. Per chunk, matmul1 does all 32 f-chunks
  (stationary w1 tile [128h,128f], moving x in two 512-token passes per k),
  relu+bias casts PSUM->fp16 into hs [128f, 32, 1024] held whole in SBUF;
  matmul2 consumes hs per 128-token tile (stationary hs [128f,128t], moving
  w2 rows) accumulating y [128t, 1024h] across all 32 f in 2 PSUM banks.
- every matmul streams 512 rows (213ns), fully shadowing each LDWEIGHTS.
- fp16 halves all traffic: x (4MiB) + w2 (8MiB) SBUF-resident, w1 streamed
  once per chunk (8MiB/chunk) on the Sync queue while bulk loads ride the
  GpSimd queue; total HBM traffic ~37MiB/core vs ~134MiB in the baseline.
- PSUM: p1 [128,1024] x2 bufs (4 banks) + y [128,1024] x2 bufs (4) = 8.
"""

import numpy as np

B, S, H, F, E, TOPK = 4, 2048, 1024, 4096, 8, 2
T = B * S
C = 2048          # per-expert device capacity: seed-0 expert loads are
#                   1932..2182; the ~291 overflow tokens (1.8% of routed
#                   pairs) run through the exact host-side fp32 fallback
#                   below, as in standard MoE capacity-factor designs (but
#                   computed exactly instead of dropped).
NF = F // 128     # 32 F-chunks
KH = H // 128     # 8 H-chunks (contraction for matmul1)
CHUNK = 1024      # tokens per chunk; hs for a full chunk stays in SBUF
NCH = C // CHUNK  # 2 chunks
NT = CHUNK // 128  # 8 token tiles per chunk

_NC_CACHE = {}


def _build_nc():
    import concourse.bacc as bacc
    import concourse.mybir as mybir
    from concourse.tile import TileContext

    f32 = mybir.dt.float32
    f16 = mybir.dt.float16
    Relu = mybir.ActivationFunctionType.Relu

    nc = bacc.Bacc("TRN2", target_bir_lowering=False, debug=False, num_devices=E,
                   dynamic_dma_scratch_size=4096)
    xt = nc.declare_dram_parameter("xt", [128, KH, C], f16, isOutput=False)
    w1t = nc.declare_dram_parameter("w1t", [128, NF, KH, 128], f16, isOutput=False)
    w2t = nc.declare_dram_parameter("w2t", [128, NF, H], f16, isOutput=False)
    b1t = nc.declare_dram_parameter("b1t", [128, NF], f32, isOutput=False)
    out = nc.declare_dram_parameter("out", [C, H], f32, isOutput=True)  # token-major

    with TileContext(nc) as tc:
        with tc.tile_pool(name="res", bufs=1) as res_pool, \
             tc.tile_pool(name="w1p", bufs=4) as w1_pool, \
             tc.tile_pool(name="hp", bufs=1) as h_pool, \
             tc.tile_pool(name="yp", bufs=3) as ysb_pool, \
             tc.tile_pool(name="p1", bufs=2, space="PSUM") as p1_pool, \
             tc.tile_pool(name="py", bufs=2, space="PSUM") as py_pool:
            xs = res_pool.tile([128, KH, C], f16)
            w2s = res_pool.tile([128, NF, H], f16)
            b1s = res_pool.tile([128, NF], f32)

            # All loads ride the two HWDGE queues (SP + Activation); the
            # GpSimd SWDGE queue returns garbage on cores 1-7 under SPMD.
            # SP queue order is arrival order: chunk-0 x halves interleaved
            # with the first w1 chunks, then per-f w1 with the background
            # items (chunk-1 x halves, w2 chunks) slotted between so each
            # w1[f] lands before the PE reaches it (~3.4us/f budget,
            # ~0.9MB/f issued).
            nc.scalar.dma_start(out=b1s[:], in_=b1t[:])
            background = [("x1", k) for k in range(KH)] + \
                         [("w2", f) for f in range(NF)]

            def pop_background(n):
                for _ in range(min(n, len(background))):
                    kind, i = background.pop(0)
                    if kind == "x1":
                        nc.sync.dma_start(out=xs[:, i, CHUNK:C],
                                          in_=xt[:, i, CHUNK:C])
                    else:
                        nc.sync.dma_start(out=w2s[:, i, :], in_=w2t[:, i, :])

            x0 = [("x0", k) for k in range(KH)]

            def pop_x0(n):
                for _ in range(min(n, len(x0))):
                    _, k = x0.pop(0)
                    nc.sync.dma_start(out=xs[:, k, 0:CHUNK], in_=xt[:, k, 0:CHUNK])

            pop_x0(2)
            for c in range(NCH):
                t0 = c * CHUNK
                # hs lives for the whole chunk: written once by m1's ACTs,
                # read 8x (once per token tile) by m2.
                hs = h_pool.tile([128, NF, CHUNK], f16, tag="hs")
                for f in range(NF):
                    w1s = w1_pool.tile([128, KH, 128], f16, tag="w1s")
                    nc.sync.dma_start(out=w1s[:], in_=w1t[:, f])
                    if c == 0:
                        if x0:
                            pop_x0(2)
                        elif f >= 4:
                            pop_background(2)
                    p1 = p1_pool.tile([128, CHUNK], f32, tag="p1")
                    for k in range(KH):
                        for h2 in range(2):
                            nc.tensor.matmul(
                                p1[:, h2 * 512:(h2 + 1) * 512], w1s[:, k, :],
                                xs[:, k, t0 + h2 * 512:t0 + (h2 + 1) * 512],
                                start=(k == 0), stop=(k == KH - 1),
                            )
                    nc.scalar.activation(hs[:, f, :], p1[:, :], Relu,
                                         bias=b1s[:, f:f + 1])
                for tt in range(NT):
                    y = py_pool.tile([128, H], f32, tag="y")
                    for f in range(NF):
                        hsl = hs[:, f, tt * 128:(tt + 1) * 128]
                        for h2 in range(2):
                            nc.tensor.matmul(
                                y[:, h2 * 512:(h2 + 1) * 512], hsl,
                                w2s[:, f, h2 * 512:(h2 + 1) * 512],
                                start=(f == 0), stop=(f == NF - 1),
                            )
                    ys = ysb_pool.tile([128, H], f32, tag="ys")
                    nc.vector.tensor_copy(ys[:], y[:])
                    nc.scalar.dma_start(
                        out=out[t0 + tt * 128:t0 + (tt + 1) * 128, :], in_=ys[:])
    nc.compile()
    return nc


def _get_nc():
    if "nc" not in _NC_CACHE:
        _NC_CACHE["nc"] = _build_nc()
    return _NC_CACHE["nc"]


def _route(xf, gate_w, gate_b):
    """Top-2 gating identical to softmax+top_k+renorm (softmax is monotonic,
    and the softmax denominator cancels in the renormalization)."""
    z = xf @ gate_w + gate_b                      # [T, E] f32
    rows = np.arange(T)
    i1 = z.argmax(1)
    z2 = z.copy()
    z2[rows, i1] = -np.inf
    i2 = z2.argmax(1)
    d = np.exp((z[rows, i2] - z[rows, i1]).astype(np.float32))
    c1 = (1.0 / (1.0 + d)).astype(np.float32)
    c2 = (1.0 - c1).astype(np.float32)
    return i1, i2, c1, c2


def _prep_expert_inputs(xf, gate_w, gate_b, w1, b1, w2, b2):
    """Gather per-expert tokens, build the per-core DRAM tensors (fp16),
    and return (in_maps, scatter, overflow)."""
    i1, i2, c1, c2 = _route(xf, gate_w, gate_b)
    in_maps, scatter, overflow = [], [], []
    for e in range(E):
        m1 = i1 == e
        m2 = i2 == e
        idx = np.concatenate([np.nonzero(m1)[0], np.nonzero(m2)[0]])
        wgt = np.concatenate([c1[m1], c2[m2]]).astype(np.float32)
        cnt = idx.size
        if cnt > C:
            # Capacity overflow (cannot happen for the fixed seed-0 inputs,
            # where the max expert load is 2182): compute the overflow rows
            # exactly on host so the result stays correct for any input.
            oidx, owgt = idx[C:], wgt[C:]
            h = np.maximum(xf[oidx] @ w1[e] + b1[e], 0.0)
            overflow.append((oidx, owgt, h @ w2[e] + b2[e]))
            idx, wgt, cnt = idx[:C], wgt[:C], C
        xg = np.zeros((C, H), np.float32)
        xg[:cnt] = xf[idx]
        xte = np.ascontiguousarray(
            xg.T.reshape(KH, 128, C).transpose(1, 0, 2)).astype(np.float16)
        w1e = np.ascontiguousarray(
            w1[e].reshape(KH, 128, NF, 128).transpose(1, 2, 0, 3)).astype(np.float16)
        w2e = np.ascontiguousarray(
            w2[e].reshape(NF, 128, H).transpose(1, 0, 2)).astype(np.float16)
        b1e = np.ascontiguousarray(b1[e].reshape(NF, 128).T)                # [128,NF]
        in_maps.append({"xt": xte, "w1t": w1e, "w2t": w2e, "b1t": b1e})
        scatter.append((idx, wgt, cnt))
    return in_maps, scatter, overflow


def kernel(x, gate_w, gate_b, w1, b1, w2, b2):
    import os
    try:  # pragma: no cover - env probe
        from antenv.axon_hooks import get_axon_ntff_profile_hook  # noqa: F401
    except ImportError:
        # BASS_TRACE=1 in the environment would send run_bass_kernel_spmd
        # down the NTFF-profiling path, which hard-imports antenv.axon_hooks.
        # If that module is absent, disable tracing rather than crash.
        os.environ.setdefault("BASS_NEVER_TRACE", "1")
    from concourse.bass_utils import run_bass_kernel_spmd

    xf = np.ascontiguousarray(np.asarray(x, dtype=np.float32).reshape(T, H))
    gate_w = np.asarray(gate_w, dtype=np.float32)
    gate_b = np.asarray(gate_b, dtype=np.float32)
    w1 = np.asarray(w1, dtype=np.float32)
    b1 = np.asarray(b1, dtype=np.float32)
    w2 = np.asarray(w2, dtype=np.float32)
    b2 = np.asarray(b2, dtype=np.float32)

    in_maps, scatter, overflow = _prep_expert_inputs(
        xf, gate_w, gate_b, w1, b1, w2, b2)

    nc = _get_nc()
    res = run_bass_kernel_spmd(nc, in_maps, core_ids=list(range(E)))

    outf = np.zeros((T, H), np.float32)
    for e in range(E):
        idx, wgt, cnt = scatter[e]
        ye = res.results[e]["out"]                                          # [C, H]
        outf[idx] += (ye[:cnt] + b2[e]) * wgt[:, None]
    for oidx, owgt, oy in overflow:
        outf[oidx] += oy * owgt[:, None]
    return outf.reshape(B, S, H)


# revision 12
# speedup vs baseline: 1.1719x; 1.1493x over previous
"""MoE layer (B=4,S=2048,H=1024,F=4096,E=8,K=2) on 8 Trainium2 NeuronCores.

Expert-parallel: the gate (0.1% of FLOPs) + top-2 routing run on host;
tokens are gathered per expert and each of the 8 cores runs one expert's
dense FFN  y = relu(x@w1+b1)@w2+b2  over its C=2048 routed tokens. The host
applies combine weights and scatter-adds the two expert contributions.

Mixed precision, chosen from measured 8-core clock behavior: with all 8
cores streaming full-rate fp16 matmuls the PE clock drops to 2.0GHz
(259ns per 512-row matmul vs 216ns single-core); fp32r (quarter the
MACs/cycle) holds 2.4GHz (227ns). So matmul1 runs in fp32r and matmul2 in
fp16 (~5e-4 rel err vs the 2e-2 gate), which also halves hs/w2 so they
stay SBUF-resident.

Structure (v4; fp32r baseline was 567us, all-fp16 v3 was 556us):
- tokens run in 2 chunks of 1024. Per chunk, matmul1 does all 32 f-chunks
  (stationary w1 [128h,128f] fp32r, moving x in two 512-token passes per
  k), relu+bias casts PSUM->fp16 into hs [128f, 32, 1024] held whole in
  SBUF; matmul2 consumes hs per 128-token tile (stationary hs [128f,128t]
  fp16, moving w2 rows fp16), accumulating y [128t, 1024h] over all 32 f
  in 2 PSUM banks. Every matmul streams 512 rows, shadowing LDWEIGHTS.
- w2 (8MiB fp16) resident; x (4MiB fp32) single chunk buffer, reloaded
  under cover of the other chunk's m2; w1 (16.8MiB fp32) streamed once
  per chunk at ~145GB/s on the SP queue; w2 rides along early. HWDGE
  queues only (SP + Activation) - GpSimd SWDGE corrupts on cores 1-7.
- PSUM: p1 [128,1024] x2 bufs (4 banks) + y [128,1024] x2 (4) = 8 exactly.
"""

import numpy as np

B, S, H, F, E, TOPK = 4, 2048, 1024, 4096, 8, 2
T = B * S
C = 2048          # per-expert device capacity: seed-0 expert loads are
#                   1932..2182; the ~291 overflow tokens (1.8% of routed
#                   pairs) run through the exact host-side fp32 fallback
#                   below, as in standard MoE capacity-factor designs (but
#                   computed exactly instead of dropped).
NF = F // 128     # 32 F-chunks
KH = H // 128     # 8 H-chunks (contraction for matmul1)
CHUNK = 1024      # tokens per chunk; hs for a full chunk stays in SBUF
NCH = C // CHUNK  # 2 chunks
NT = CHUNK // 128  # 8 token tiles per chunk

_NC_CACHE = {}


def _build_nc():
    import concourse.bacc as bacc
    import concourse.mybir as mybir
    from concourse.tile import TileContext

    f32 = mybir.dt.float32
    f32r = mybir.dt.float32r
    f16 = mybir.dt.float16
    Relu = mybir.ActivationFunctionType.Relu

    nc = bacc.Bacc("TRN2", target_bir_lowering=False, debug=False, num_devices=E,
                   dynamic_dma_scratch_size=4096)
    xt = nc.declare_dram_parameter("xt", [128, KH, C], f32r, isOutput=False)
    w1t = nc.declare_dram_parameter("w1t", [128, NF, KH, 128], f32r, isOutput=False)
    w2t = nc.declare_dram_parameter("w2t", [128, NF, H], f16, isOutput=False)
    b1t = nc.declare_dram_parameter("b1t", [128, NF], f32, isOutput=False)
    out = nc.declare_dram_parameter("out", [C, H], f32, isOutput=True)  # token-major

    with TileContext(nc) as tc:
        with tc.tile_pool(name="res", bufs=1) as res_pool, \
             tc.tile_pool(name="xp", bufs=1) as x_pool, \
             tc.tile_pool(name="w1p", bufs=4) as w1_pool, \
             tc.tile_pool(name="hp", bufs=1) as h_pool, \
             tc.tile_pool(name="yp", bufs=3) as ysb_pool, \
             tc.tile_pool(name="p1", bufs=2, space="PSUM") as p1_pool, \
             tc.tile_pool(name="py", bufs=2, space="PSUM") as py_pool:
            w2s = res_pool.tile([128, NF, H], f16)
            b1s = res_pool.tile([128, NF], f32)
            nc.scalar.dma_start(out=b1s[:], in_=b1t[:])

            # w2 chunks ride the SP queue between w1 chunks during chunk-0
            # m1; they are only needed when m2 starts (~115us in).
            background = [("w2", f) for f in range(NF)]

            def pop_background(n):
                for _ in range(min(n, len(background))):
                    _, i = background.pop(0)
                    nc.sync.dma_start(out=w2s[:, i, :], in_=w2t[:, i, :])

            for c in range(NCH):
                t0 = c * CHUNK
                # x chunk buffer (fp32): chunk 1's reload happens under
                # cover of chunk 0's m2 (WAR on the single buffer delays
                # the DMA until m1(c0) finished reading).
                xs = x_pool.tile([128, KH, CHUNK], f32r, tag="xs")
                # hs lives for the whole chunk: written once by m1's ACTs,
                # read 8x (once per token tile) by m2.
                hs = h_pool.tile([128, NF, CHUNK], f16, tag="hs")
                for k in range(KH):
                    nc.sync.dma_start(out=xs[:, k, :], in_=xt[:, k, t0:t0 + CHUNK])
                for f in range(NF):
                    w1s = w1_pool.tile([128, KH, 128], f32r, tag="w1s")
                    nc.sync.dma_start(out=w1s[:], in_=w1t[:, f])
                    if c == 0 and f >= 2:
                        pop_background(2)
                    p1 = p1_pool.tile([128, CHUNK], f32, tag="p1")
                    for k in range(KH):
                        for h2 in range(2):
                            nc.tensor.matmul(
                                p1[:, h2 * 512:(h2 + 1) * 512], w1s[:, k, :],
                                xs[:, k, h2 * 512:(h2 + 1) * 512],
                                start=(k == 0), stop=(k == KH - 1),
                            )
                    nc.scalar.activation(hs[:, f, :], p1[:, :], Relu,
                                         bias=b1s[:, f:f + 1])
                for tt in range(NT):
                    y = py_pool.tile([128, H], f32, tag="y")
                    for f in range(NF):
                        hsl = hs[:, f, tt * 128:(tt + 1) * 128]
                        for h2 in range(2):
                            nc.tensor.matmul(
                                y[:, h2 * 512:(h2 + 1) * 512], hsl,
                                w2s[:, f, h2 * 512:(h2 + 1) * 512],
                                start=(f == 0), stop=(f == NF - 1),
                            )
                    # copy+store in halves so the final tile's drain
                    # pipelines the copy with the DMA
                    trow = out[t0 + tt * 128:t0 + (tt + 1) * 128, :]
                    for h2 in range(2):
                        ys = ysb_pool.tile([128, 512], f32, tag="ys")
                        nc.vector.tensor_copy(ys[:], y[:, h2 * 512:(h2 + 1) * 512])
                        nc.scalar.dma_start(
                            out=trow[:, h2 * 512:(h2 + 1) * 512], in_=ys[:])
    nc.compile()
    return nc


def _get_nc():
    if "nc" not in _NC_CACHE:
        _NC_CACHE["nc"] = _build_nc()
    return _NC_CACHE["nc"]


def _route(xf, gate_w, gate_b):
    """Top-2 gating identical to softmax+top_k+renorm (softmax is monotonic,
    and the softmax denominator cancels in the renormalization)."""
    z = xf @ gate_w + gate_b                      # [T, E] f32
    rows = np.arange(T)
    i1 = z.argmax(1)
    z2 = z.copy()
    z2[rows, i1] = -np.inf
    i2 = z2.argmax(1)
    d = np.exp((z[rows, i2] - z[rows, i1]).astype(np.float32))
    c1 = (1.0 / (1.0 + d)).astype(np.float32)
    c2 = (1.0 - c1).astype(np.float32)
    return i1, i2, c1, c2


def _prep_expert_inputs(xf, gate_w, gate_b, w1, b1, w2, b2):
    """Gather per-expert tokens, build the per-core DRAM tensors, and
    return (in_maps, scatter, overflow)."""
    i1, i2, c1, c2 = _route(xf, gate_w, gate_b)
    in_maps, scatter, overflow = [], [], []
    for e in range(E):
        m1 = i1 == e
        m2 = i2 == e
        idx = np.concatenate([np.nonzero(m1)[0], np.nonzero(m2)[0]])
        wgt = np.concatenate([c1[m1], c2[m2]]).astype(np.float32)
        cnt = idx.size
        if cnt > C:
            # Capacity overflow (cannot happen for the fixed seed-0 inputs,
            # where the max expert load is 2182): compute the overflow rows
            # exactly on host so the result stays correct for any input.
            oidx, owgt = idx[C:], wgt[C:]
            h = np.maximum(xf[oidx] @ w1[e] + b1[e], 0.0)
            overflow.append((oidx, owgt, h @ w2[e] + b2[e]))
            idx, wgt, cnt = idx[:C], wgt[:C], C
        xg = np.zeros((C, H), np.float32)
        xg[:cnt] = xf[idx]
        xte = np.ascontiguousarray(
            xg.T.reshape(KH, 128, C).transpose(1, 0, 2))                    # [128,KH,C] f32
        w1e = np.ascontiguousarray(
            w1[e].reshape(KH, 128, NF, 128).transpose(1, 2, 0, 3))          # [128,NF,KH,128] f32
        w2e = np.ascontiguousarray(
            w2[e].reshape(NF, 128, H).transpose(1, 0, 2)).astype(np.float16)
        b1e = np.ascontiguousarray(b1[e].reshape(NF, 128).T)                # [128,NF]
        in_maps.append({"xt": xte, "w1t": w1e, "w2t": w2e, "b1t": b1e})
        scatter.append((idx, wgt, cnt))
    return in_maps, scatter, overflow


def kernel(x, gate_w, gate_b, w1, b1, w2, b2):
    import os
    try:  # pragma: no cover - env probe
        from antenv.axon_hooks import get_axon_ntff_profile_hook  # noqa: F401
    except ImportError:
        # BASS_TRACE=1 in the environment would send run_bass_kernel_spmd
        # down the NTFF-profiling path, which hard-imports antenv.axon_hooks.
        # If that module is absent, disable tracing rather than crash.
        os.environ.setdefault("BASS_NEVER_TRACE", "1")
    from concourse.bass_utils import run_bass_kernel_spmd

    xf = np.ascontiguousarray(np.asarray(x, dtype=np.float32).reshape(T, H))
    gate_w = np.asarray(gate_w, dtype=np.float32)
    gate_b = np.asarray(gate_b, dtype=np.float32)
    w1 = np.asarray(w1, dtype=np.float32)
    b1 = np.asarray(b1, dtype=np.float32)
    w2 = np.asarray(w2, dtype=np.float32)
    b2 = np.asarray(b2, dtype=np.float32)

    in_maps, scatter, overflow = _prep_expert_inputs(
        xf, gate_w, gate_b, w1, b1, w2, b2)

    nc = _get_nc()
    res = run_bass_kernel_spmd(nc, in_maps, core_ids=list(range(E)))

    outf = np.zeros((T, H), np.float32)
    for e in range(E):
        idx, wgt, cnt = scatter[e]
        ye = res.results[e]["out"]                                          # [C, H]
        outf[idx] += (ye[:cnt] + b2[e]) * wgt[:, None]
    for oidx, owgt, oy in overflow:
        outf[oidx] += oy * owgt[:, None]
    return outf.reshape(B, S, H)


# revision 17
# speedup vs baseline: 1.1818x; 1.0084x over previous
"""MoE layer (B=4,S=2048,H=1024,F=4096,E=8,K=2) on 8 Trainium2 NeuronCores.

Expert-parallel: the gate (0.1% of FLOPs) + top-2 routing run on host;
tokens are gathered per expert and each of the 8 cores runs one expert's
dense FFN  y = relu(x@w1+b1)@w2+b2  over its C=2048 routed tokens. The host
applies combine weights and scatter-adds the two expert contributions.

Mixed precision, chosen from measured 8-core clock behavior: with all 8
cores streaming full-rate fp16 matmuls the PE clock drops to 2.0GHz
(259ns per 512-row matmul vs 216ns single-core); fp32r (quarter the
MACs/cycle) holds 2.4GHz (227ns). So matmul1 runs in fp32r and matmul2 in
fp16 (~5e-4 rel err vs the 2e-2 gate), which also halves hs/w2 so they
stay SBUF-resident.

Structure (v4; fp32r baseline was 567us, all-fp16 v3 was 556us):
- tokens run in 2 chunks of 1024. Per chunk, matmul1 does all 32 f-chunks
  (stationary w1 [128h,128f] fp32r, moving x in two 512-token passes per
  k), relu+bias casts PSUM->fp16 into hs [128f, 32, 1024] held whole in
  SBUF; matmul2 consumes hs per 128-token tile (stationary hs [128f,128t]
  fp16, moving w2 rows fp16), accumulating y [128t, 1024h] over all 32 f
  in 2 PSUM banks. Every matmul streams 512 rows, shadowing LDWEIGHTS.
- w2 (8MiB fp16) resident; x (4MiB fp32) single chunk buffer, reloaded
  under cover of the other chunk's m2; w1 (16.8MiB fp32) streamed once
  per chunk at ~145GB/s on the SP queue; w2 rides along early. HWDGE
  queues only (SP + Activation) - GpSimd SWDGE corrupts on cores 1-7.
- PSUM: p1 [128,1024] x2 bufs (4 banks) + y [128,1024] x2 (4) = 8 exactly.
"""

import numpy as np

B, S, H, F, E, TOPK = 4, 2048, 1024, 4096, 8, 2
T = B * S
C = 2048          # per-expert device capacity: seed-0 expert loads are
#                   1932..2182; the ~291 overflow tokens (1.8% of routed
#                   pairs) run through the exact host-side fp32 fallback
#                   below, as in standard MoE capacity-factor designs (but
#                   computed exactly instead of dropped).
NF = F // 128     # 32 F-chunks
KH = H // 128     # 8 H-chunks (contraction for matmul1)
CHUNK = 1024      # tokens per chunk; hs for a full chunk stays in SBUF
NCH = C // CHUNK  # 2 chunks
NT = CHUNK // 128  # 8 token tiles per chunk

_NC_CACHE = {}


def _build_nc():
    import concourse.bacc as bacc
    import concourse.mybir as mybir
    from concourse.tile import TileContext

    f32 = mybir.dt.float32
    f32r = mybir.dt.float32r
    f16 = mybir.dt.float16
    Relu = mybir.ActivationFunctionType.Relu

    nc = bacc.Bacc("TRN2", target_bir_lowering=False, debug=False, num_devices=E,
                   dynamic_dma_scratch_size=4096)
    xt = nc.declare_dram_parameter("xt", [128, KH, C], f16, isOutput=False)
    w1t = nc.declare_dram_parameter("w1t", [128, NF, KH, 128], f32r, isOutput=False)
    w2t = nc.declare_dram_parameter("w2t", [128, NF, H], f16, isOutput=False)
    b1t = nc.declare_dram_parameter("b1t", [128, NF], f32, isOutput=False)
    out = nc.declare_dram_parameter("out", [C, H], f32, isOutput=True)  # token-major

    with TileContext(nc) as tc:
        with tc.tile_pool(name="res", bufs=1) as res_pool, \
             tc.tile_pool(name="xp", bufs=1) as x_pool, \
             tc.tile_pool(name="w1p", bufs=4) as w1_pool, \
             tc.tile_pool(name="hp", bufs=1) as h_pool, \
             tc.tile_pool(name="yp", bufs=3) as ysb_pool, \
             tc.tile_pool(name="p1", bufs=2, space="PSUM") as p1_pool, \
             tc.tile_pool(name="py", bufs=2, space="PSUM") as py_pool:
            w2s = res_pool.tile([128, NF, H], f16)
            b1s = res_pool.tile([128, NF], f32)
            nc.scalar.dma_start(out=b1s[:], in_=b1t[:])

            # w2 chunks ride the SP queue between w1 chunks during chunk-0
            # m1; they are only needed when m2 starts (~115us in).
            background = [("w2", f) for f in range(NF)]

            def pop_background(n):
                for _ in range(min(n, len(background))):
                    _, i = background.pop(0)
                    nc.sync.dma_start(out=w2s[:, i, :], in_=w2t[:, i, :])

            w1_head = {}
            for c in range(NCH):
                t0 = c * CHUNK
                # x streams in as fp16 (halving the startup-critical fill)
                # and is cast per k-slice to fp32 on the vector engine —
                # exact, only the initial fp16 rounding of x is lost
                # (~5e-4, irrelevant vs the 2e-2 gate). Chunk 1's reload
                # happens under cover of chunk 0's m2 (WAR on the single
                # buffer delays the DMA until m1(c0) finished reading).
                xs16 = x_pool.tile([128, KH, CHUNK], f16, tag="xs16")
                xs = x_pool.tile([128, KH, CHUNK], f32r, tag="xs")
                # hs lives for the whole chunk: written once by m1's ACTs,
                # read 8x (once per token tile) by m2.
                hs = h_pool.tile([128, NF, CHUNK], f16, tag="hs")
                if c == 0:
                    # w1[0..2] ahead of x in the queue so f=0 can begin the
                    # moment the first x slices land
                    for f in range(3):
                        w1_head[f] = w1_pool.tile([128, KH, 128], f32r,
                                                  tag="w1s", name=f"w1h{f}")
                        nc.sync.dma_start(out=w1_head[f][:], in_=w1t[:, f])
                for k in range(KH):
                    nc.sync.dma_start(out=xs16[:, k, :], in_=xt[:, k, t0:t0 + CHUNK])
                    nc.vector.tensor_copy(xs[:, k, :], xs16[:, k, :])
                for f in range(NF):
                    if c == 0 and f < 3:
                        w1s = w1_head[f]
                    else:
                        w1s = w1_pool.tile([128, KH, 128], f32r, tag="w1s")
                        nc.sync.dma_start(out=w1s[:], in_=w1t[:, f])
                    if c == 0 and f >= 2:
                        pop_background(2)
                    p1 = p1_pool.tile([128, CHUNK], f32, tag="p1")
                    for k in range(KH):
                        for h2 in range(2):
                            nc.tensor.matmul(
                                p1[:, h2 * 512:(h2 + 1) * 512], w1s[:, k, :],
                                xs[:, k, h2 * 512:(h2 + 1) * 512],
                                start=(k == 0), stop=(k == KH - 1),
                            )
                    nc.scalar.activation(hs[:, f, :], p1[:, :], Relu,
                                         bias=b1s[:, f:f + 1])
                for tt in range(NT):
                    y = py_pool.tile([128, H], f32, tag="y")
                    for f in range(NF):
                        hsl = hs[:, f, tt * 128:(tt + 1) * 128]
                        for h2 in range(2):
                            nc.tensor.matmul(
                                y[:, h2 * 512:(h2 + 1) * 512], hsl,
                                w2s[:, f, h2 * 512:(h2 + 1) * 512],
                                start=(f == 0), stop=(f == NF - 1),
                            )
                    # copy+store in pieces so the final tile's drain
                    # pipelines the copy with the DMA; quarters on the very
                    # last tile shorten the kernel's tail
                    trow = out[t0 + tt * 128:t0 + (tt + 1) * 128, :]
                    last = (c == NCH - 1) and (tt == NT - 1)
                    qn, qw = (4, 256) if last else (2, 512)
                    for q in range(qn):
                        ys = ysb_pool.tile([128, 512], f32, tag="ys")
                        nc.vector.tensor_copy(ys[:, :qw],
                                              y[:, q * qw:(q + 1) * qw])
                        nc.scalar.dma_start(
                            out=trow[:, q * qw:(q + 1) * qw], in_=ys[:, :qw])
    nc.compile()
    return nc


def _get_nc():
    if "nc" not in _NC_CACHE:
        _NC_CACHE["nc"] = _build_nc()
    return _NC_CACHE["nc"]


def _route(xf, gate_w, gate_b):
    """Top-2 gating identical to softmax+top_k+renorm (softmax is monotonic,
    and the softmax denominator cancels in the renormalization)."""
    z = xf @ gate_w + gate_b                      # [T, E] f32
    rows = np.arange(T)
    i1 = z.argmax(1)
    z2 = z.copy()
    z2[rows, i1] = -np.inf
    i2 = z2.argmax(1)
    d = np.exp((z[rows, i2] - z[rows, i1]).astype(np.float32))
    c1 = (1.0 / (1.0 + d)).astype(np.float32)
    c2 = (1.0 - c1).astype(np.float32)
    return i1, i2, c1, c2


def _prep_expert_inputs(xf, gate_w, gate_b, w1, b1, w2, b2):
    """Gather per-expert tokens, build the per-core DRAM tensors, and
    return (in_maps, scatter, overflow)."""
    i1, i2, c1, c2 = _route(xf, gate_w, gate_b)
    in_maps, scatter, overflow = [], [], []
    for e in range(E):
        m1 = i1 == e
        m2 = i2 == e
        idx = np.concatenate([np.nonzero(m1)[0], np.nonzero(m2)[0]])
        wgt = np.concatenate([c1[m1], c2[m2]]).astype(np.float32)
        cnt = idx.size
        if cnt > C:
            # Capacity overflow (cannot happen for the fixed seed-0 inputs,
            # where the max expert load is 2182): compute the overflow rows
            # exactly on host so the result stays correct for any input.
            oidx, owgt = idx[C:], wgt[C:]
            h = np.maximum(xf[oidx] @ w1[e] + b1[e], 0.0)
            overflow.append((oidx, owgt, h @ w2[e] + b2[e]))
            idx, wgt, cnt = idx[:C], wgt[:C], C
        xg = np.zeros((C, H), np.float32)
        xg[:cnt] = xf[idx]
        xte = np.ascontiguousarray(
            xg.T.reshape(KH, 128, C).transpose(1, 0, 2)).astype(np.float16)
        w1e = np.ascontiguousarray(
            w1[e].reshape(KH, 128, NF, 128).transpose(1, 2, 0, 3))          # [128,NF,KH,128] f32
        w2e = np.ascontiguousarray(
            w2[e].reshape(NF, 128, H).transpose(1, 0, 2)).astype(np.float16)
        b1e = np.ascontiguousarray(b1[e].reshape(NF, 128).T)                # [128,NF]
        in_maps.append({"xt": xte, "w1t": w1e, "w2t": w2e, "b1t": b1e})
        scatter.append((idx, wgt, cnt))
    return in_maps, scatter, overflow


def kernel(x, gate_w, gate_b, w1, b1, w2, b2):
    import os
    try:  # pragma: no cover - env probe
        from antenv.axon_hooks import get_axon_ntff_profile_hook  # noqa: F401
    except ImportError:
        # BASS_TRACE=1 in the environment would send run_bass_kernel_spmd
        # down the NTFF-profiling path, which hard-imports antenv.axon_hooks.
        # If that module is absent, disable tracing rather than crash.
        os.environ.setdefault("BASS_NEVER_TRACE", "1")
    from concourse.bass_utils import run_bass_kernel_spmd

    xf = np.ascontiguousarray(np.asarray(x, dtype=np.float32).reshape(T, H))
    gate_w = np.asarray(gate_w, dtype=np.float32)
    gate_b = np.asarray(gate_b, dtype=np.float32)
    w1 = np.asarray(w1, dtype=np.float32)
    b1 = np.asarray(b1, dtype=np.float32)
    w2 = np.asarray(w2, dtype=np.float32)
    b2 = np.asarray(b2, dtype=np.float32)

    in_maps, scatter, overflow = _prep_expert_inputs(
        xf, gate_w, gate_b, w1, b1, w2, b2)

    nc = _get_nc()
    res = run_bass_kernel_spmd(nc, in_maps, core_ids=list(range(E)))

    outf = np.zeros((T, H), np.float32)
    for e in range(E):
        idx, wgt, cnt = scatter[e]
        ye = res.results[e]["out"]                                          # [C, H]
        outf[idx] += (ye[:cnt] + b2[e]) * wgt[:, None]
    for oidx, owgt, oy in overflow:
        outf[oidx] += oy * owgt[:, None]
    return outf.reshape(B, S, H)
